# revision 42
# baseline (speedup 1.0000x reference)
"""Trainium2 Bass kernel for the Cocoa contrastive loss.

loss = mean_i exp((1 - cos(x_i, y_i))/tau)
     + sum_{i in neg, j not in neg} exp(cos(x_i, x_j)/tau) / cnt
     + sum_{i in neg, j not in neg} exp(cos(y_i, y_j)/tau) / cnt

with neg = rows whose label has > 32 zeros, cnt = n_neg * n_nonneg.

The only O(B^2 * D) compute — the two masked Gram GEMMs with exp-sum — runs
on the 8 NeuronCores.  Everything that is O(B*D) preprocessing or O(B)
postprocessing (neg mask, row permutation, l2-normalize, fp8 quantization,
operand transposes/layouts, the pos term, final combine) happens on the
host, where it costs well under a second and keeps the device launch at
the GEMM roofline.

Device launch (4x2 core grid over neg x nonneg rows): per-core fp8
DoubleRow GEMM sim = A_neg @ B_nonneg^T with K=D on partitions,
exp(sim/tau) on ScalarE with per-partition accumulation; returns
[128, n_blocks] partial sums per core.  The host subtracts the exp(0)=1
contributions of the zero padding and divides by cnt.

fp8 quantization (scale 24/|row|, centering N(0, 1/4096) rows in e4m3's
normal range) puts ~2e-4 relative error on the result, far inside the
2e-2 gate.
"""

import numpy as np
import ml_dtypes

import concourse.bass as bass
import concourse.bacc as bacc
import concourse.mybir as mybir
import concourse.tile as tile
from concourse.bass_utils import run_bass_kernel_spmd

TAU = 0.1
THRESHOLD = 32
B, D, L = 4096, 4096, 64
NCORES = 8
KCH = D // 128      # 32 contraction chunks
A_SPLIT, B_SPLIT = 4, 2  # core grid over (neg rows, nonneg rows)

F32 = mybir.dt.float32
BF16 = mybir.dt.bfloat16
FP8 = mybir.dt.float8e4
FP8_NP = ml_dtypes.float8_e4m3fn
FP8_SCALE = 24.0

# module-level caches so repeated kernel() calls don't rebuild/recompile
_CACHE: dict = {}

# filled in by the last kernel() call when tracing is enabled (test harness use)
LAST_RESULTS: list = []


def _build_phase2(m_loc: int, n_loc: int) -> bass.Bass:
    """Per-core fp8 DoubleRow GEMM: [m_loc neg rows] x [n_loc nonneg rows].

    Operand roles are swapped vs the natural orientation: the nonneg side is
    the 128-wide stationary operand and the neg side is the 512-wide moving
    operand, so the matmul stream (~220ns) fully hides LDWEIGHTS (~142ns).
    Host-supplied layouts (fully contiguous per DMA):
      l{x,y}: [128, KCH, m_loc]        moving side (neg rows)
      r{x,y}: [n_ch, 128, KCH, 128]    stationary side (nonneg rows)
    """
    nc = bacc.Bacc(None)
    n_ch = n_loc // 128
    n_ms = -(-m_loc // 512)  # moving sub-tiles of <=512
    assert m_loc % 16 == 0 and n_loc % 128 == 0
    lx = nc.declare_dram_parameter("lx", [128, KCH, m_loc], FP8, isOutput=False)
    rx = nc.declare_dram_parameter("rx", [n_ch, 128, KCH, 128], FP8, isOutput=False)
    ly = nc.declare_dram_parameter("ly", [128, KCH, m_loc], FP8, isOutput=False)
    ry = nc.declare_dram_parameter("ry", [n_ch, 128, KCH, 128], FP8, isOutput=False)
    acc_out = nc.declare_dram_parameter("acc", [128, 2 * n_ch * n_ms], F32,
                                        isOutput=True)

    msizes = [min(512, m_loc - 512 * i) for i in range(n_ms)]

    with tile.TileContext(nc) as tc:
        with (
            tc.tile_pool(name="mov", bufs=1) as movp,
            tc.tile_pool(name="sta", bufs=8) as stap,
            tc.tile_pool(name="ps", bufs=8, space="PSUM") as psp,
            tc.tile_pool(name="junk", bufs=4) as junkp,
            tc.tile_pool(name="accp", bufs=1) as accp,
        ):
            acc = accp.tile([128, 2 * n_ch * n_ms], F32)
            # (A PE p-state warmup with dummy matmuls during the operand
            # wait was measured NEUTRAL-to-worse: the ramp phase is
            # delivery-bound — the GEMM's x half finishes exactly when its
            # operands finish arriving — so the slow early tensor clock is
            # hidden behind DMA waits anyway.)
            # The DMA fabric drains issues in order, so issue strictly in
            # consumption order with the first GEMM block's pieces
            # interleaved finely: kp0-7 of block 0 unblock after ~0.75MiB.
            # ly is consumed only after the whole x half (~45us in);
            # issuing it earlier steals early-delivery bandwidth from
            # operands the PE waits on (measured +2..5us).  Consolidating
            # the per-nch stationary tiles into big multi-nch DMAs was
            # also measured WORSE (+2.7us): coarse transfers gate several
            # blocks at once and delay the final DMA-ring drain before
            # teardown.
            lt = {}
            st = {}
            st["x", 0] = stap.tile([128, KCH, 128], FP8, tag="st", name="st_x0")
            lt["x"] = movp.tile([128, KCH, m_loc], FP8, tag="lx", name="lt_x")
            # single sync-queue issue only: with two HWDGE rings in flight
            # the fabric interleaves them and first-needed operands finish
            # last (measured +3.8us ramp when lx pieces went to the scalar
            # queue).  One ring preserves strict consumption-order delivery.
            # Deliver the first GROUP's operands so ready work tracks the
            # kp-major consumption: interleave st_x0-3's halves between
            # lx's quarters, so after each lx piece lands the PE has 4
            # blocks' worth of kp-work instead of 1 (block-major order
            # left 7us of early matmul waits with only st_x0 resident).
            nfirst = min(4, n_ch)
            for nch in range(1, nfirst):
                st["x", nch] = stap.tile([128, KCH, 128], FP8, tag="st",
                                         name=f"st_x{nch}")
            h, q = KCH // 2, KCH // 4
            nc.sync.dma_start(out=st["x", 0][:, :h, :], in_=rx[0, :, :h, :])
            nc.sync.dma_start(out=lt["x"][:, :q, :], in_=lx[:, :q, :])
            for nch in range(1, nfirst):
                nc.sync.dma_start(out=st["x", nch][:, :h, :],
                                  in_=rx[nch, :, :h, :])
            nc.sync.dma_start(out=lt["x"][:, q:h, :], in_=lx[:, q:h, :])
            for nch in range(nfirst):
                nc.sync.dma_start(out=st["x", nch][:, h:, :],
                                  in_=rx[nch, :, h:, :])
            nc.sync.dma_start(out=lt["x"][:, h:3 * q, :], in_=lx[:, h:3 * q, :])
            nc.sync.dma_start(out=lt["x"][:, 3 * q:, :], in_=lx[:, 3 * q:, :])
            lt["y"] = movp.tile([128, KCH, m_loc], FP8, tag="ly", name="lt_y")
            nc.sync.dma_start(out=lt["y"], in_=ly[:])

            # kp-major interleave across groups of 4 blocks (4 PSUM banks):
            # block 0 alone spans ~7.7us of delivery waits for lx's pieces,
            # while blocks 1-3's stationary tiles are already resident —
            # interleaving their kp-work hides their compute under block
            # 0's delivery gate instead of serializing it after.
            units = [(name, nch, ms)
                     for name in ("x", "y")
                     for nch in range(n_ch)
                     for ms in range(n_ms)]
            col_of = {u: i for i, u in enumerate(units)}
            GROUP = 4
            for gi in range(0, len(units), GROUP):
                grp = units[gi:gi + GROUP]
                pss = {}
                for u in grp:
                    name, nch, ms = u
                    if (name, nch) not in st:
                        st[name, nch] = stap.tile([128, KCH, 128], FP8,
                                                  tag="st",
                                                  name=f"st_{name}{nch}")
                        rsrc = rx if name == "x" else ry
                        nc.sync.dma_start(out=st[name, nch], in_=rsrc[nch])
                    pss[u] = psp.tile([128, 512], F32, tag="ps",
                                      name=f"ps_{name}{nch}_{ms}")
                for kp in range(KCH // 2):
                    for u in grp:
                        name, nch, ms = u
                        msz = msizes[ms]
                        nc.tensor.matmul(
                            pss[u][:, :msz],
                            lhsT=st[name, nch][:, 2 * kp:2 * kp + 2, :],
                            rhs=lt[name][:, 2 * kp:2 * kp + 2,
                                         512 * ms:512 * ms + msz],
                            start=(kp == 0), stop=(kp == KCH // 2 - 1),
                            perf_mode=mybir.MatmulPerfMode.DoubleRow)
                for u in grp:
                    name, nch, ms = u
                    msz = msizes[ms]
                    j = junkp.tile([128, 512], BF16, tag="junk")
                    nc.scalar.activation(
                        j[:, :msz], pss[u][:, :msz],
                        mybir.ActivationFunctionType.Exp,
                        scale=1.0 / (TAU * FP8_SCALE * FP8_SCALE),
                        accum_out=acc[:, col_of[u]:col_of[u] + 1])
            nc.sync.dma_start(out=acc_out[:], in_=acc)
    nc.compile()
    return nc


def _run_spmd(key, builder, in_maps):
    import os
    if key not in _CACHE:
        _CACHE[key] = builder()
    nc = _CACHE[key]
    trace = bool(os.environ.get("COCOA_TRACE"))
    res = run_bass_kernel_spmd(nc, in_maps, list(range(NCORES)), trace=trace)
    LAST_RESULTS.append((key, res))
    return res.results


def kernel(x_pred_batch: np.ndarray, y_pred_batch: np.ndarray,
           label_batch: np.ndarray) -> np.ndarray:
    x = np.ascontiguousarray(x_pred_batch, dtype=np.float32)
    y = np.ascontiguousarray(y_pred_batch, dtype=np.float32)
    lab = np.asarray(label_batch)

    # exact mask / permutation bookkeeping
    zero_counts = (lab == 0).sum(axis=1)
    neg_mask = zero_counts > THRESHOLD
    idx = np.concatenate([np.flatnonzero(neg_mask), np.flatnonzero(~neg_mask)])
    n1 = int(neg_mask.sum())
    n2 = B - n1
    cnt = n1 * n2

    # l2-normalize, scale into e4m3's range, quantize (host preprocessing)
    xq = (x * (FP8_SCALE / np.sqrt(np.einsum('bd,bd->b', x, x)))[:, None]
          ).astype(FP8_NP)
    yq = (y * (FP8_SCALE / np.sqrt(np.einsum('bd,bd->b', y, y)))[:, None]
          ).astype(FP8_NP)

    # pos term from the quantized embeddings, in float64
    cos_pos = np.einsum('bd,bd->b', xq.astype(np.float32),
                        yq.astype(np.float32)).astype(np.float64)
    cos_pos /= FP8_SCALE * FP8_SCALE
    pos_error = float(np.mean(np.exp((1.0 - cos_pos) / TAU)))

    neg_total = 0.0
    if cnt > 0:
        m_loc = 16 * max(1, -(-n1 // (A_SPLIT * 16)))
        n_loc = 128 * max(1, -(-n2 // (B_SPLIT * 128)))
        n1p, n2p = A_SPLIT * m_loc, B_SPLIT * n_loc
        n_ch = n_loc // 128
        n_ms = -(-m_loc // 512)

        padded = {}
        for nm, t in (("x", xq), ("y", yq)):
            # [128, KCH, B]: tt[p, c, r] = t[perm[r], c*128 + p]
            tt = t[idx].T.reshape(KCH, 128, B).transpose(1, 0, 2)
            lhs = np.zeros((128, KCH, n1p), FP8_NP)
            lhs[:, :, :n1] = tt[:, :, :n1]
            rhs = np.zeros((128, KCH, n2p), FP8_NP)
            rhs[:, :, :n2] = tt[:, :, n1:]
            padded["l" + nm] = lhs
            padded["r" + nm] = np.ascontiguousarray(
                rhs.reshape(128, KCH, B_SPLIT * n_ch, 128).transpose(2, 0, 1, 3))

        in_maps = []
        for c in range(NCORES):
            a, bgrid = divmod(c, B_SPLIT)
            cmap = {}
            for nm in ("x", "y"):
                cmap["l" + nm] = np.ascontiguousarray(
                    padded["l" + nm][:, :, a * m_loc:(a + 1) * m_loc])
                cmap["r" + nm] = padded["r" + nm][bgrid * n_ch:(bgrid + 1) * n_ch]
            in_maps.append(cmap)

        res = _run_spmd(("phase2v17", m_loc, n_loc),
                        lambda: _build_phase2(m_loc, n_loc), in_maps)

        n_half = n_ch * n_ms
        sx = sy = 0.0
        for r in res:
            acc = r["acc"].astype(np.float64)
            sx += acc[:, :n_half].sum()
            sy += acc[:, n_half:].sum()
        pad = float(n1p) * n2p - float(n1) * n2
        neg_total = ((sx - pad) + (sy - pad)) / cnt

    return np.float32(pos_error + neg_total)


# revision 44
# speedup vs baseline: 1.0344x; 1.0344x over previous
"""Trainium2 Bass kernel for the Cocoa contrastive loss.

loss = mean_i exp((1 - cos(x_i, y_i))/tau)
     + sum_{i in neg, j not in neg} exp(cos(x_i, x_j)/tau) / cnt
     + sum_{i in neg, j not in neg} exp(cos(y_i, y_j)/tau) / cnt

with neg = rows whose label has > 32 zeros, cnt = n_neg * n_nonneg.

The only O(B^2 * D) compute — the two masked Gram GEMMs with exp-sum — runs
on the 8 NeuronCores.  Everything that is O(B*D) preprocessing or O(B)
postprocessing (neg mask, row permutation, l2-normalize, fp8 quantization,
operand transposes/layouts, the pos term, final combine) happens on the
host, where it costs well under a second and keeps the device launch at
the GEMM roofline.

Device launch (4x2 core grid over neg x nonneg rows): per-core fp8
DoubleRow GEMM sim = A_neg @ B_nonneg^T with K=D on partitions,
exp(sim/tau) on ScalarE with per-partition accumulation; returns
[128, n_blocks] partial sums per core.  The host subtracts the exp(0)=1
contributions of the zero padding and divides by cnt.

fp8 quantization (scale 24/|row|, centering N(0, 1/4096) rows in e4m3's
normal range) puts ~2e-4 relative error on the result, far inside the
2e-2 gate.
"""

import numpy as np
import ml_dtypes

import concourse.bass as bass
import concourse.bacc as bacc
import concourse.mybir as mybir
import concourse.tile as tile
from concourse.bass_utils import run_bass_kernel_spmd

TAU = 0.1
THRESHOLD = 32
B, D, L = 4096, 4096, 64
NCORES = 8
KCH = D // 128      # 32 contraction chunks
A_SPLIT, B_SPLIT = 4, 2  # core grid over (neg rows, nonneg rows)

F32 = mybir.dt.float32
BF16 = mybir.dt.bfloat16
FP8 = mybir.dt.float8e4
FP8_NP = ml_dtypes.float8_e4m3fn
FP8_SCALE = 24.0

# module-level caches so repeated kernel() calls don't rebuild/recompile
_CACHE: dict = {}

# filled in by the last kernel() call when tracing is enabled (test harness use)
LAST_RESULTS: list = []


def _build_phase2(m_loc: int, n_loc: int) -> bass.Bass:
    """Per-core fp8 DoubleRow GEMM: [m_loc neg rows] x [n_loc nonneg rows].

    Operand roles are swapped vs the natural orientation: the nonneg side is
    the 128-wide stationary operand and the neg side is the 512-wide moving
    operand, so the matmul stream (~220ns) fully hides LDWEIGHTS (~142ns).
    Host-supplied layouts (fully contiguous per DMA):
      l{x,y}: [128, KCH, m_loc]        moving side (neg rows)
      r{x,y}: [n_ch, 128, KCH, 128]    stationary side (nonneg rows)
    """
    nc = bacc.Bacc(None)
    n_ch = n_loc // 128
    n_ms = -(-m_loc // 512)  # moving sub-tiles of <=512
    assert m_loc % 16 == 0 and n_loc % 128 == 0
    lx = nc.declare_dram_parameter("lx", [128, KCH, m_loc], FP8, isOutput=False)
    rx = nc.declare_dram_parameter("rx", [n_ch, 128, KCH, 128], FP8, isOutput=False)
    ly = nc.declare_dram_parameter("ly", [128, KCH, m_loc], FP8, isOutput=False)
    ry = nc.declare_dram_parameter("ry", [n_ch, 128, KCH, 128], FP8, isOutput=False)
    acc_out = nc.declare_dram_parameter("acc", [128, 2 * n_ch * n_ms], F32,
                                        isOutput=True)

    msizes = [min(512, m_loc - 512 * i) for i in range(n_ms)]

    with tile.TileContext(nc) as tc:
        with (
            tc.tile_pool(name="mov", bufs=1) as movp,
            tc.tile_pool(name="sta", bufs=8) as stap,
            tc.tile_pool(name="ps", bufs=8, space="PSUM") as psp,
            tc.tile_pool(name="junk", bufs=4) as junkp,
            tc.tile_pool(name="accp", bufs=1) as accp,
        ):
            acc = accp.tile([128, 2 * n_ch * n_ms], F32)
            # (A PE p-state warmup with dummy matmuls during the operand
            # wait was measured NEUTRAL-to-worse: the ramp phase is
            # delivery-bound — the GEMM's x half finishes exactly when its
            # operands finish arriving — so the slow early tensor clock is
            # hidden behind DMA waits anyway.)
            # The DMA fabric drains issues in order, so issue strictly in
            # consumption order with the first GEMM block's pieces
            # interleaved finely: kp0-7 of block 0 unblock after ~0.75MiB.
            # ly is consumed only after the whole x half (~45us in);
            # issuing it earlier steals early-delivery bandwidth from
            # operands the PE waits on (measured +2..5us).  Consolidating
            # the per-nch stationary tiles into big multi-nch DMAs was
            # also measured WORSE (+2.7us): coarse transfers gate several
            # blocks at once and delay the final DMA-ring drain before
            # teardown.
            lt = {}
            st = {}
            st["x", 0] = stap.tile([128, KCH, 128], FP8, tag="st", name="st_x0")
            lt["x"] = movp.tile([128, KCH, m_loc], FP8, tag="lx", name="lt_x")
            # single sync-queue issue only: with two HWDGE rings in flight
            # the fabric interleaves them and first-needed operands finish
            # last (measured +3.8us ramp when lx pieces went to the scalar
            # queue).  One ring preserves strict consumption-order delivery.
            # (Interleaving st_x1-3 half-loads between lx's quarters to
            # front-load 4 blocks of kp-ready work was measured WORSE
            # (+4.4us): the extra small DMAs stack per-transfer fixed
            # latency in the fabric's spin-up window, delaying lx's tail
            # pieces that gate every kp>=8.)
            nc.sync.dma_start(out=st["x", 0][:, :KCH // 2, :],
                              in_=rx[0, :, :KCH // 2, :])
            nc.sync.dma_start(out=lt["x"][:, :KCH // 4, :], in_=lx[:, :KCH // 4, :])
            nc.sync.dma_start(out=st["x", 0][:, KCH // 2:, :],
                              in_=rx[0, :, KCH // 2:, :])
            nc.sync.dma_start(out=lt["x"][:, KCH // 4:KCH // 2, :],
                              in_=lx[:, KCH // 4:KCH // 2, :])
            nc.sync.dma_start(out=lt["x"][:, KCH // 2:, :], in_=lx[:, KCH // 2:, :])
            for nch in range(1, min(4, n_ch)):
                st["x", nch] = stap.tile([128, KCH, 128], FP8, tag="st",
                                         name=f"st_x{nch}")
                nc.sync.dma_start(out=st["x", nch], in_=rx[nch])
            lt["y"] = movp.tile([128, KCH, m_loc], FP8, tag="ly", name="lt_y")
            nc.sync.dma_start(out=lt["y"], in_=ly[:])

            # kp-major interleave across groups of 4 blocks (4 PSUM banks):
            # block 0 alone spans ~7.7us of delivery waits for lx's pieces,
            # while blocks 1-3's stationary tiles are already resident —
            # interleaving their kp-work hides their compute under block
            # 0's delivery gate instead of serializing it after.
            units = [(name, nch, ms)
                     for name in ("x", "y")
                     for nch in range(n_ch)
                     for ms in range(n_ms)]
            col_of = {u: i for i, u in enumerate(units)}
            GROUP = 4
            for gi in range(0, len(units), GROUP):
                grp = units[gi:gi + GROUP]
                pss = {}
                for u in grp:
                    name, nch, ms = u
                    if (name, nch) not in st:
                        st[name, nch] = stap.tile([128, KCH, 128], FP8,
                                                  tag="st",
                                                  name=f"st_{name}{nch}")
                        rsrc = rx if name == "x" else ry
                        nc.sync.dma_start(out=st[name, nch], in_=rsrc[nch])
                    pss[u] = psp.tile([128, 512], F32, tag="ps",
                                      name=f"ps_{name}{nch}_{ms}")
                for kp in range(KCH // 2):
                    for u in grp:
                        name, nch, ms = u
                        msz = msizes[ms]
                        nc.tensor.matmul(
                            pss[u][:, :msz],
                            lhsT=st[name, nch][:, 2 * kp:2 * kp + 2, :],
                            rhs=lt[name][:, 2 * kp:2 * kp + 2,
                                         512 * ms:512 * ms + msz],
                            start=(kp == 0), stop=(kp == KCH // 2 - 1),
                            perf_mode=mybir.MatmulPerfMode.DoubleRow)
                for u in grp:
                    name, nch, ms = u
                    msz = msizes[ms]
                    j = junkp.tile([128, 512], BF16, tag="junk")
                    nc.scalar.activation(
                        j[:, :msz], pss[u][:, :msz],
                        mybir.ActivationFunctionType.Exp,
                        scale=1.0 / (TAU * FP8_SCALE * FP8_SCALE),
                        accum_out=acc[:, col_of[u]:col_of[u] + 1])
            nc.sync.dma_start(out=acc_out[:], in_=acc)
    nc.compile()
    return nc


def _run_spmd(key, builder, in_maps):
    import os
    if key not in _CACHE:
        _CACHE[key] = builder()
    nc = _CACHE[key]
    trace = bool(os.environ.get("COCOA_TRACE"))
    res = run_bass_kernel_spmd(nc, in_maps, list(range(NCORES)), trace=trace)
    LAST_RESULTS.append((key, res))
    return res.results


def kernel(x_pred_batch: np.ndarray, y_pred_batch: np.ndarray,
           label_batch: np.ndarray) -> np.ndarray:
    x = np.ascontiguousarray(x_pred_batch, dtype=np.float32)
    y = np.ascontiguousarray(y_pred_batch, dtype=np.float32)
    lab = np.asarray(label_batch)

    # exact mask / permutation bookkeeping
    zero_counts = (lab == 0).sum(axis=1)
    neg_mask = zero_counts > THRESHOLD
    idx = np.concatenate([np.flatnonzero(neg_mask), np.flatnonzero(~neg_mask)])
    n1 = int(neg_mask.sum())
    n2 = B - n1
    cnt = n1 * n2

    # l2-normalize, scale into e4m3's range, quantize (host preprocessing)
    xq = (x * (FP8_SCALE / np.sqrt(np.einsum('bd,bd->b', x, x)))[:, None]
          ).astype(FP8_NP)
    yq = (y * (FP8_SCALE / np.sqrt(np.einsum('bd,bd->b', y, y)))[:, None]
          ).astype(FP8_NP)

    # pos term from the quantized embeddings, in float64
    cos_pos = np.einsum('bd,bd->b', xq.astype(np.float32),
                        yq.astype(np.float32)).astype(np.float64)
    cos_pos /= FP8_SCALE * FP8_SCALE
    pos_error = float(np.mean(np.exp((1.0 - cos_pos) / TAU)))

    neg_total = 0.0
    if cnt > 0:
        m_loc = 16 * max(1, -(-n1 // (A_SPLIT * 16)))
        n_loc = 128 * max(1, -(-n2 // (B_SPLIT * 128)))
        n1p, n2p = A_SPLIT * m_loc, B_SPLIT * n_loc
        n_ch = n_loc // 128
        n_ms = -(-m_loc // 512)

        padded = {}
        for nm, t in (("x", xq), ("y", yq)):
            # [128, KCH, B]: tt[p, c, r] = t[perm[r], c*128 + p]
            tt = t[idx].T.reshape(KCH, 128, B).transpose(1, 0, 2)
            lhs = np.zeros((128, KCH, n1p), FP8_NP)
            lhs[:, :, :n1] = tt[:, :, :n1]
            rhs = np.zeros((128, KCH, n2p), FP8_NP)
            rhs[:, :, :n2] = tt[:, :, n1:]
            padded["l" + nm] = lhs
            padded["r" + nm] = np.ascontiguousarray(
                rhs.reshape(128, KCH, B_SPLIT * n_ch, 128).transpose(2, 0, 1, 3))

        in_maps = []
        for c in range(NCORES):
            a, bgrid = divmod(c, B_SPLIT)
            cmap = {}
            for nm in ("x", "y"):
                cmap["l" + nm] = np.ascontiguousarray(
                    padded["l" + nm][:, :, a * m_loc:(a + 1) * m_loc])
                cmap["r" + nm] = padded["r" + nm][bgrid * n_ch:(bgrid + 1) * n_ch]
            in_maps.append(cmap)

        res = _run_spmd(("phase2v18", m_loc, n_loc),
                        lambda: _build_phase2(m_loc, n_loc), in_maps)

        n_half = n_ch * n_ms
        sx = sy = 0.0
        for r in res:
            acc = r["acc"].astype(np.float64)
            sx += acc[:, :n_half].sum()
            sy += acc[:, n_half:].sum()
        pad = float(n1p) * n2p - float(n1) * n2
        neg_total = ((sx - pad) + (sy - pad)) / cnt

    return np.float32(pos_error + neg_total)
